# revision 41
# baseline (speedup 1.0000x reference)
"""Causal single-head attention with per-batch length masking, on 8 trn2 cores.

Problem: x[8,2048,1024] f32, Wq/Wk/Wv[1024,64] f32, lengths[8] int64.
  q,k,v = x@W*;  s = q@k^T (causal + length-pair mask, -inf);  s *= H^-0.5
  out = softmax(s) @ v          -> [8, 2048, 64] f32

Math note: for row i < len: every causal key j<=i is also valid, so the
pair-mask never bites -> plain causal softmax. For row i >= len: only the
diagonal survives -> out[i] = v[i]. Instead of blending, LARGE is added to
the diagonal score of masked rows (via the same bias-matmul that applies
the causal mask), so softmax itself collapses to ~1 on the diagonal and
out[i] = v[i] falls out of the normal normalize path.

Sharding: data-parallel over batch, one batch element per NeuronCore.

Perf structure (v3):
  - matmuls pipeline on the PE at ~N*0.417ns once deps are ready; levers
    are startup, dependency stalls (in-order engine queues!), and tail.
  - emission order is software-pipelined: pv(m) is emitted after s(m+1)
    so the in-order PE queue never head-of-line blocks on the ACT exp;
    the next chunk's projection is split (qk | v) around the last pv.
  - DMA issue split across Sync+Scalar HWDGE queues, ordered by need;
    mid-kernel DMAs stay on Sync (Scalar is the ACT/exp queue).
  - ~24 dummy ident matmuls flip the PE HAM clock gate during DMA wait.
  - po psum->sbuf copies run on ACT (Copy activation); DVE was the
    second-busiest engine.
  - last chunk: po split in half, out path per half + final pair split,
    so the post-matmul tail is short.
"""

import sys

import numpy as np
import ml_dtypes

try:
    import concourse.bass as bass  # noqa: F401
except ImportError:
    sys.path.insert(0, "/opt/trn_rl_repo")

import concourse.bass as bass  # noqa: F401
import concourse.mybir as mybir
import concourse.tile as tile
from concourse import bacc
from concourse.bass_utils import run_bass_kernel_spmd
from concourse.masks import make_identity, make_lower_triangular

F32 = mybir.dt.float32
BF16 = mybir.dt.bfloat16
BF16_NP = ml_dtypes.bfloat16

B, T, E, H = 8, 2048, 1024, 64
HP = H + 2       # rows of po read back (out + denom col)
P = 128          # partitions
CH = 512         # i-chunk width
ET = E // P      # 8 e-tiles
NCH = T // CH    # 4 chunks
NIT = T // P     # 16 i-tiles
SCALE = float(H) ** -0.5
LARGE = 600.0    # masked-row diagonal boost; exp(600/8)=3.7e32 stays finite
NWARM = 20       # dummy matmuls to flip the PE HAM clock gate


def build_nc():
    nc = bacc.Bacc(
        "TRN2",
        target_bir_lowering=False,
        debug=False,
        num_devices=B,
    )

    xb_d = nc.dram_tensor("xb", [P, NCH, ET, CH], BF16, kind="ExternalInput").ap()
    wqk_d = nc.dram_tensor("wqk", [P, ET * 2 * H], BF16, kind="ExternalInput").ap()
    # wv [P, ET*H] then iml [P, NIT] packed on the free axis
    wvim_d = nc.dram_tensor(
        "wvim", [P, ET * H + NIT], BF16, kind="ExternalInput"
    ).ap()
    out_d = nc.dram_tensor("out", [T, H], F32, kind="ExternalOutput").ap()

    with tile.TileContext(nc) as tc:
        with (
            tc.tile_pool(name="const", bufs=1) as cpool,
            tc.tile_pool(name="xt", bufs=1) as xtpool,
            tc.tile_pool(name="qk", bufs=1) as qkpool,
            tc.tile_pool(name="v", bufs=1) as vpool,
            tc.tile_pool(name="mq", bufs=16) as mqpool,
            tc.tile_pool(name="stage", bufs=4) as stpool,
            tc.tile_pool(name="pt", bufs=4) as ptpool,
            tc.tile_pool(name="blend", bufs=4) as blpool,
            tc.tile_pool(name="ps_s", bufs=2, space="PSUM") as ps_s,
            tc.tile_pool(name="ps_o", bufs=2, space="PSUM") as ps_o,
            tc.tile_pool(name="ps_m", bufs=2, space="PSUM") as ps_m,
        ):
            # ---- DMA: split across sync + scalar queues, ordered by when
            # the data is first consumed.
            wqk_all = cpool.tile([P, ET * 2 * H], BF16, tag="wqk")
            xt_all = xtpool.tile([P, NCH * ET * CH], BF16, tag="xt")
            wvim_all = cpool.tile([P, ET * H + NIT], BF16, tag="wvim")

            def xt_sl(e, c):
                return xt_all[:, (c * ET + e) * CH : (c * ET + e + 1) * CH]

            def xt_chunk(c, e0, e1):
                return xt_all[
                    :, (c * ET + e0) * CH : (c * ET + e1) * CH
                ].rearrange("p (n w) -> p n w", w=CH)

            # x streams in 2-e-tile (256KB) pieces alternating across both
            # HWDGE queues in global consumption order, so projections are
            # gated at fine grain and compute starts as early as possible
            # (early DMA BW paces the first half of the kernel).
            # weights ride the scalar ring so the sync ring's FIRST transfer
            # is x e0-e1 -- the first matmul gates on max(wqk, e01) landing
            # in parallel instead of their sum on one ring.
            nc.scalar.dma_start(out=wqk_all[:], in_=wqk_d[:])
            nc.scalar.dma_start(out=wvim_all[:], in_=wvim_d[:])
            # the scalar queue is ALSO the ACT/exp queue: every DMA issue
            # parked there delays the first exp by ~650ns. It carries only
            # the early transfers (weights + 2 chunk-0 pieces, needed
            # before any exp can run anyway); chunks 1-3 ride sync alone.
            for c in range(NCH):
                for p_ in range(4):
                    eng = nc.scalar if (c == 0 and p_ % 2 == 1) else nc.sync
                    eng.dma_start(
                        out=xt_chunk(c, 2 * p_, 2 * p_ + 2),
                        in_=xb_d[:, c, 2 * p_ : 2 * p_ + 2, :],
                    )

            wqk_sb = [wqk_all[:, e * 2 * H : (e + 1) * 2 * H] for e in range(ET)]
            wv_sb = [wvim_all[:, e * H : (e + 1) * H] for e in range(ET)]
            iml_b = wvim_all[:, ET * H : ET * H + NIT]

            # ---- constants ----
            ident = cpool.tile([P, P], F32, tag="ident")
            make_identity(nc, ident[:])
            ident_b = cpool.tile([P, P], BF16, tag="ident_b")
            nc.vector.tensor_copy(ident_b[:], ident[:])
            # strict lower-triangular -1e30 (mask sT where j > i within block)
            ltri = cpool.tile([P, P], F32, tag="ltri")
            make_lower_triangular(nc, ltri[:], val=-1e30, diag=False)
            ltri_b = cpool.tile([P, P], BF16, tag="ltri_b")
            nc.vector.tensor_copy(ltri_b[:], ltri[:])

            # prewarm the ACT exp table off the critical path
            warm = cpool.tile([1, 2], F32, tag="warm")
            nc.gpsimd.memset(warm[:], 0.0)
            nc.scalar.activation(
                warm[:, 0:1], warm[:, 1:2], mybir.ActivationFunctionType.Exp
            )

            # PE warm-up: dummy matmuls on the identity flip the HAM clock
            # gate to 8/8 while the x DMA is still in flight.
            pwarm = ps_m.tile([P, P], F32, tag="pm", name="pwarm")
            for w in range(NWARM):
                nc.tensor.matmul(
                    pwarm[:], ident_b[:], ident_b[:], start=True, stop=True
                )

            # persistent per-chunk tiles; v holds 4 i-tiles x [128, 128]
            # (padded cols; col 64 of each block is the ones-column -> denom)
            qt_sb = [None] * NCH
            kt_sb = [None] * NCH
            v_sb = []
            for c in range(NCH):
                vt = vpool.tile([P, 4 * P], BF16, tag=f"v{c}")
                nc.gpsimd.memset(vt[:], 1.0)
                v_sb.append(vt)

            def v_blk(it):
                return v_sb[it // 4][:, (it % 4) * P : (it % 4) * P + P]

            def qt(c):
                return qt_sb[c][:]

            def kt_blk(j):
                return kt_sb[j // 4][:, (j % 4) * P : (j % 4 + 1) * P]

            # M[it] = ltri + diag(iml[:, it]): causal mask + masked-row boost
            m_sb = [None] * NIT
            po_sb = [None] * NCH

            def emit_mask(it):
                mt = mqpool.tile([P, P], BF16, tag="m", name=f"m{it}")
                nc.vector.scalar_tensor_tensor(
                    mt[:],
                    ident_b[:],
                    iml_b[:, it : it + 1],
                    ltri_b[:],
                    op0=mybir.AluOpType.mult,
                    op1=mybir.AluOpType.add,
                )
                m_sb[it] = mt

            def emit_proj_qk(c):
                # q/k projection (fused): psum[0:64]=qT, [64:128]=kT
                pqk = ps_m.tile([P, CH], F32, tag="pm", name="pqk")
                for e in range(ET):
                    nc.tensor.matmul(
                        pqk[:],
                        wqk_sb[e],
                        xt_sl(e, c),
                        start=(e == 0),
                        stop=(e == ET - 1),
                    )
                qtt = qkpool.tile([H, CH], BF16, tag=f"qt{c}")
                nc.vector.tensor_copy(qtt[:], pqk[0:H, :])
                qt_sb[c] = qtt
                ktt = qkpool.tile([H, CH], BF16, tag=f"kt{c}")
                nc.vector.tensor_copy(ktt[:], pqk[H : 2 * H, :])
                kt_sb[c] = ktt
                if c == 0:
                    # first two diag masks are needed by the very first
                    # score pair
                    emit_mask(0)
                    emit_mask(1)

            def emit_proj_v(c):
                # v projection (vT), then PE-transpose to v [t,h]
                pv = ps_m.tile([H, CH], F32, tag="pm", name="pv")
                for e in range(ET):
                    nc.tensor.matmul(
                        pv[:],
                        wv_sb[e],
                        xt_sl(e, c),
                        start=(e == 0),
                        stop=(e == ET - 1),
                    )
                vt_st = stpool.tile([H, CH], BF16, tag="vt")
                nc.vector.tensor_copy(vt_st[:], pv[:])
                pvt = ps_m.tile([P, 4 * H], BF16, tag="pm", name="pvt")
                for k in range(4):
                    nc.tensor.transpose(
                        pvt[:, k * H : (k + 1) * H],
                        vt_st[:, k * P : (k + 1) * P],
                        ident_b[0:H, 0:H],
                    )
                nc.vector.tensor_copy(
                    v_sb[c][:].rearrange("p (k w) -> p k w", w=P)[:, :, 0:H],
                    pvt[:].rearrange("p (k w) -> p k w", w=H),
                )
                # diag masks for this chunk's i-tiles (the c=0 first two are
                # emitted early in emit_proj_qk)
                for k in range(4):
                    it = c * 4 + k
                    if m_sb[it] is None:
                        emit_mask(it)

            def chunk_pairs(c):
                # full-tile pairs in natural order; the four diag tiles
                # are re-paired with the LARGER-offset tile in slot 0 so
                # the joint exp read window [offs0 .. 1024) has (almost)
                # no dead gap columns -- exp is the pacer of this phase.
                jd = 4 * c
                pairs = [(2 * m, 2 * m + 1) for m in range(2 * c)]
                if c == NCH - 1:
                    # keep i-tiles 12/13 finishing in the first diag pair
                    # so the poA half out path still overlaps the last pair
                    pairs += [(jd + 1, jd), (jd + 3, jd + 2)]
                else:
                    pairs += [(jd + 3, jd), (jd + 2, jd + 1)]
                return pairs

            def emit_s(c, pair, pss):
                # score matmuls for one j-pair of chunk c; returns offs
                offs = []
                for s, j in enumerate(pair):
                    off = max(0, j * P - c * CH)
                    offs.append(off)
                    full = j < 4 * c
                    nc.tensor.matmul(
                        pss[:, s * CH + off : (s + 1) * CH],
                        kt_blk(j),
                        qt(c)[:, off:CH],
                        start=True,
                        stop=full,
                    )
                    if not full:
                        nc.tensor.matmul(
                            pss[:, s * CH + off : s * CH + off + P],
                            ident_b[:],
                            m_sb[j][:],
                            start=False,
                            stop=True,
                        )
                return offs

            def emit_pv(c, pair, pt, offs, split_po, last_j, lastA_j=None):
                # ascending-j emission: j=0 (the start=True full-region
                # write) always lands before any accumulating overlap
                for s in sorted(range(2), key=lambda s_: pair[s_]):
                    j = pair[s]
                    off = offs[s]
                    if not split_po:
                        nc.tensor.matmul(
                            po_sb[c][:, off:CH],
                            v_blk(j),
                            pt[:, s * CH + off : (s + 1) * CH],
                            start=(j == 0),
                            stop=(j == last_j),
                        )
                    else:
                        HC = CH // 2
                        poA, poB = po_sb[c]
                        if off < HC:
                            nc.tensor.matmul(
                                poA[:, off:HC],
                                v_blk(j),
                                pt[:, s * CH + off : s * CH + HC],
                                start=(j == 0),
                                stop=(j == lastA_j),
                            )
                        ob_ = max(off, HC)
                        nc.tensor.matmul(
                            poB[:, ob_ - HC : HC],
                            v_blk(j),
                            pt[:, s * CH + ob_ : (s + 1) * CH],
                            start=(j == 0),
                            stop=(j == last_j),
                        )

            def emit_out_copy(c):
                ot = stpool.tile([HP, CH], BF16, tag="ot")
                nc.vector.tensor_copy(ot[:], po_sb[c][0:HP, :])
                return ot

            def emit_out_rest(c, ot):
                # transpose 4 i-tiles into one psum tile, then batched
                # normalize (masked rows come out as v via the LARGE boost).
                # pot lives in ps_o (not ps_m) so the next chunk's
                # projection psum never waits on this out-path's reads.
                pot = ps_o.tile([P, 4 * HP], BF16, tag="po", name="pot")
                for k in range(4):
                    nc.tensor.transpose(
                        pot[:, k * HP : (k + 1) * HP],
                        ot[:, k * P : (k + 1) * P],
                        ident_b[0:HP, 0:HP],
                    )
                pot3 = pot[:].rearrange("p (k w) -> p k w", w=HP)
                recip = blpool.tile([P, 4], F32, tag="recip")
                nc.vector.reciprocal(
                    recip[:].rearrange("p (k o) -> p k o", o=1),
                    pot3[:, :, H : H + 1],
                )
                ob = blpool.tile([P, 4 * H], F32, tag="ob")
                for k in range(4):
                    nc.vector.tensor_scalar_mul(
                        ob[:, k * H : (k + 1) * H],
                        pot3[:, k, 0:H],
                        recip[:, k : k + 1],
                    )
                nc.sync.dma_start(
                    out=out_d.rearrange("(n p) h -> p n h", p=P)[
                        :, c * 4 : (c + 1) * 4, :
                    ],
                    in_=ob[:].rearrange("p (n h) -> p n h", h=H),
                )

            def emit_out_last_half(c, po_half, h0):
                # out path for i-tiles [c*4+h0*2, c*4+h0*2+2) of the last
                # chunk (po was split in two psum tiles)
                ot = stpool.tile([HP, CH // 2], BF16, tag="oth")
                nc.vector.tensor_copy(ot[:], po_half[0:HP, :])
                pot = ps_m.tile([P, 2 * HP], BF16, tag="pm", name=f"poth{h0}")
                for k in range(2):
                    nc.tensor.transpose(
                        pot[:, k * HP : (k + 1) * HP],
                        ot[:, k * P : (k + 1) * P],
                        ident_b[0:HP, 0:HP],
                    )
                pot3 = pot[:].rearrange("p (k w) -> p k w", w=HP)
                recip = blpool.tile([P, 2], F32, tag="reciph")
                nc.vector.reciprocal(
                    recip[:].rearrange("p (k o) -> p k o", o=1),
                    pot3[:, :, H : H + 1],
                )
                ob = blpool.tile([P, 2 * H], F32, tag="obh")
                for k in range(2):
                    nc.vector.tensor_scalar_mul(
                        ob[:, k * H : (k + 1) * H],
                        pot3[:, k, 0:H],
                        recip[:, k : k + 1],
                    )
                # the last half rides the scalar queue (exp is done by then)
                # so it doesn't serialize behind the first half's transfer
                eng = nc.scalar if h0 == 1 else nc.sync
                eng.dma_start(
                    out=out_d.rearrange("(n p) h -> p n h", p=P)[
                        :, c * 4 + h0 * 2 : c * 4 + h0 * 2 + 2, :
                    ],
                    in_=ob[:].rearrange("p (n h) -> p n h", h=H),
                )

            # ---- main loop (simple per-pair emission measured fastest;
            # skewed/interleaved variants consistently lost ~1-2us) ----
            # the (c-1) output path is emitted AFTER the chunk-c projection
            # so PE never stalls on the DVE copies
            # chunk-boundary drought fix: the next chunk's qk-projection is
            # emitted MID-pair-loop (so qt/kt are ready the moment the
            # boundary is crossed and ACT never starves waiting for the
            # first scores), and this chunk's v-projection is deferred into
            # its own pair loop (v(c) is only read by the last two pairs).
            emit_proj_qk(0)
            emit_proj_v(0)
            ot_prev = None
            for c in range(NCH):
                split_po = c == NCH - 1
                njt = 4 * c + 4
                if split_po:
                    # out_rest(c-1) must allocate its pot BEFORE poA/poB,
                    # else the ps_o slot rotation makes pot wait on poA's
                    # read which sits behind it in the in-order PE queue
                    if ot_prev is not None:
                        emit_out_rest(c - 1, ot_prev)
                        ot_prev = None
                    poA = ps_o.tile([P, CH // 2], F32, tag="po", name="poA")
                    poB = ps_o.tile([P, CH // 2], F32, tag="po", name="poB")
                    po_sb[c] = (poA, poB)
                else:
                    po_sb[c] = ps_o.tile([P, CH], F32, tag="po", name="po")
                pairs = chunk_pairs(c)
                last_j = 15 if split_po else 4 * c + 2
                # pv lags 1-2 pairs behind the scores: pv(k-d) sits AFTER
                # s(k) in the in-order PE queue, so its wait on exp never
                # delays the dispatch of the next scores. Depth 2 only for
                # the late chunks (their x landed long ago; a deep pend
                # behind DMA-gated work traps the pipeline).
                depth = 1 if c < 2 else 2
                pend = []
                for pi, pair in enumerate(pairs):
                    pss = ps_s.tile([P, 2 * CH], F32, tag="ps")
                    offs = emit_s(c, pair, pss)
                    pt = ptpool.tile([P, 2 * CH], BF16, tag="pt")
                    nc.scalar.activation(
                        pt[:, offs[0] :], pss[:, offs[0] :],
                        mybir.ActivationFunctionType.Exp,
                        scale=SCALE,
                    )
                    if pi == 1 and c >= 1:
                        # this chunk's v-projection: only its diag pairs
                        # (pi >= 2) read v(c); deferring it one pair lets
                        # the chunk's first exps dispatch sooner
                        emit_proj_v(c)
                    if pi == 2 and ot_prev is not None:
                        emit_out_rest(c - 1, ot_prev)
                        ot_prev = None
                    if split_po and pi == len(pairs) - 1:
                        # poA (i-tiles 12,13) needs pair (13,12)'s pv;
                        # drain after the last pair's s/exp so the poA
                        # transposes never delay the scores dispatch
                        while pend:
                            emit_pv(c, *pend.pop(0), split_po, last_j, 13)
                        emit_out_last_half(c, poA, 0)
                    elif len(pend) >= depth:
                        emit_pv(c, *pend.pop(0), split_po, last_j, 13)
                    pend.append((pair, pt, offs))
                    if pi == len(pairs) - 2 and c + 1 < NCH:
                        emit_proj_qk(c + 1)
                while pend:
                    emit_pv(c, *pend.pop(0), split_po, last_j, 13)
                if not split_po:
                    ot_prev = emit_out_copy(c)
                else:
                    emit_out_last_half(c, poB, 1)

    nc.compile()
    return nc


_NC_CACHE = None


def _get_nc():
    global _NC_CACHE
    if _NC_CACHE is None:
        _NC_CACHE = build_nc()
    return _NC_CACHE


def make_in_maps(x, Wq, Wk, Wv, lengths):
    wqk = np.concatenate(
        [np.asarray(Wq, dtype=np.float32), np.asarray(Wk, dtype=np.float32)],
        axis=1,
    )
    # [E, F] -> [P, ET*F]
    wqk_t = np.ascontiguousarray(
        wqk.reshape(ET, P, 2 * H).transpose(1, 0, 2).reshape(P, ET * 2 * H)
    ).astype(BF16_NP)
    wv_t = (
        np.asarray(Wv, dtype=np.float32)
        .reshape(ET, P, H)
        .transpose(1, 0, 2)
        .reshape(P, ET * H)
    )
    in_maps = []
    for b in range(B):
        # x[b] [T, E] -> xb [P, NCH, ET, CH]:  xb[p,c,e,w] = x[c*CH+w, e*P+p]
        xb = np.ascontiguousarray(
            np.asarray(x[b], dtype=np.float32)
            .reshape(NCH, CH, ET, P)
            .transpose(3, 0, 2, 1)
        ).astype(BF16_NP)
        mflat = (np.arange(T) < int(lengths[b])).astype(np.float32)
        iml = (1.0 - mflat.reshape(NIT, P).T) * LARGE  # [128, 16]
        wvim = np.ascontiguousarray(
            np.concatenate([wv_t, iml], axis=1)
        ).astype(BF16_NP)
        in_maps.append({"xb": xb, "wqk": wqk_t, "wvim": wvim})
    return in_maps


def run(x, Wq, Wk, Wv, lengths, trace=False):
    nc = _get_nc()
    in_maps = make_in_maps(x, Wq, Wk, Wv, lengths)
    res = run_bass_kernel_spmd(
        nc, in_maps, core_ids=list(range(B)), trace=trace
    )
    out = np.stack([res.results[b]["out"] for b in range(B)], axis=0)
    return out, res


def kernel(x, Wq, Wk, Wv, lengths):
    try:
        out, _ = run(x, Wq, Wk, Wv, lengths, trace=False)
    except Exception:
        # the device occasionally wedges (NRT_EXEC_UNIT_UNRECOVERABLE);
        # request a core reset and retry once
        import os

        os.environ["NEURON_RT_RESET_CORES"] = "1"
        out, _ = run(x, Wq, Wk, Wv, lengths, trace=False)
    return out


# revision 43
# speedup vs baseline: 1.0123x; 1.0123x over previous
"""Causal single-head attention with per-batch length masking, on 8 trn2 cores.

Problem: x[8,2048,1024] f32, Wq/Wk/Wv[1024,64] f32, lengths[8] int64.
  q,k,v = x@W*;  s = q@k^T (causal + length-pair mask, -inf);  s *= H^-0.5
  out = softmax(s) @ v          -> [8, 2048, 64] f32

Math note: for row i < len: every causal key j<=i is also valid, so the
pair-mask never bites -> plain causal softmax. For row i >= len: only the
diagonal survives -> out[i] = v[i]. Instead of blending, LARGE is added to
the diagonal score of masked rows (via the same bias-matmul that applies
the causal mask), so softmax itself collapses to ~1 on the diagonal and
out[i] = v[i] falls out of the normal normalize path.

Sharding: data-parallel over batch, one batch element per NeuronCore.

Perf structure (v3):
  - matmuls pipeline on the PE at ~N*0.417ns once deps are ready; levers
    are startup, dependency stalls (in-order engine queues!), and tail.
  - emission order is software-pipelined: pv(m) is emitted after s(m+1)
    so the in-order PE queue never head-of-line blocks on the ACT exp;
    the next chunk's projection is split (qk | v) around the last pv.
  - DMA issue split across Sync+Scalar HWDGE queues, ordered by need;
    mid-kernel DMAs stay on Sync (Scalar is the ACT/exp queue).
  - ~24 dummy ident matmuls flip the PE HAM clock gate during DMA wait.
  - po psum->sbuf copies run on ACT (Copy activation); DVE was the
    second-busiest engine.
  - last chunk: po split in half, out path per half + final pair split,
    so the post-matmul tail is short.
"""

import sys

import numpy as np
import ml_dtypes

try:
    import concourse.bass as bass  # noqa: F401
except ImportError:
    sys.path.insert(0, "/opt/trn_rl_repo")

import concourse.bass as bass  # noqa: F401
import concourse.mybir as mybir
import concourse.tile as tile
from concourse import bacc
from concourse.bass_utils import run_bass_kernel_spmd
from concourse.masks import make_identity, make_lower_triangular

F32 = mybir.dt.float32
BF16 = mybir.dt.bfloat16
BF16_NP = ml_dtypes.bfloat16

B, T, E, H = 8, 2048, 1024, 64
HP = H + 2       # rows of po read back (out + denom col)
P = 128          # partitions
CH = 512         # i-chunk width
ET = E // P      # 8 e-tiles
NCH = T // CH    # 4 chunks
NIT = T // P     # 16 i-tiles
SCALE = float(H) ** -0.5
LARGE = 600.0    # masked-row diagonal boost; exp(600/8)=3.7e32 stays finite
NWARM = 24       # dummy matmuls to flip the PE HAM clock gate


def build_nc():
    nc = bacc.Bacc(
        "TRN2",
        target_bir_lowering=False,
        debug=False,
        num_devices=B,
    )

    xb_d = nc.dram_tensor("xb", [P, NCH, ET, CH], BF16, kind="ExternalInput").ap()
    wqk_d = nc.dram_tensor("wqk", [P, ET * 2 * H], BF16, kind="ExternalInput").ap()
    # wv [P, ET*H] then iml [P, NIT] packed on the free axis
    wvim_d = nc.dram_tensor(
        "wvim", [P, ET * H + NIT], BF16, kind="ExternalInput"
    ).ap()
    out_d = nc.dram_tensor("out", [T, H], F32, kind="ExternalOutput").ap()

    with tile.TileContext(nc) as tc:
        with (
            tc.tile_pool(name="const", bufs=1) as cpool,
            tc.tile_pool(name="xt", bufs=1) as xtpool,
            tc.tile_pool(name="qk", bufs=1) as qkpool,
            tc.tile_pool(name="v", bufs=1) as vpool,
            tc.tile_pool(name="mq", bufs=16) as mqpool,
            tc.tile_pool(name="stage", bufs=4) as stpool,
            tc.tile_pool(name="pt", bufs=4) as ptpool,
            tc.tile_pool(name="blend", bufs=4) as blpool,
            tc.tile_pool(name="ps_s", bufs=2, space="PSUM") as ps_s,
            tc.tile_pool(name="ps_o", bufs=2, space="PSUM") as ps_o,
            tc.tile_pool(name="ps_m", bufs=2, space="PSUM") as ps_m,
        ):
            # ---- DMA: split across sync + scalar queues, ordered by when
            # the data is first consumed.
            wqk_all = cpool.tile([P, ET * 2 * H], BF16, tag="wqk")
            xt_all = xtpool.tile([P, NCH * ET * CH], BF16, tag="xt")
            wvim_all = cpool.tile([P, ET * H + NIT], BF16, tag="wvim")

            def xt_sl(e, c):
                return xt_all[:, (c * ET + e) * CH : (c * ET + e + 1) * CH]

            def xt_chunk(c, e0, e1):
                return xt_all[
                    :, (c * ET + e0) * CH : (c * ET + e1) * CH
                ].rearrange("p (n w) -> p n w", w=CH)

            # x streams in 2-e-tile (256KB) pieces alternating across both
            # HWDGE queues in global consumption order, so projections are
            # gated at fine grain and compute starts as early as possible
            # (early DMA BW paces the first half of the kernel).
            # weights ride the scalar ring so the sync ring's FIRST transfer
            # is x e0-e1 -- the first matmul gates on max(wqk, e01) landing
            # in parallel instead of their sum on one ring.
            nc.scalar.dma_start(out=wqk_all[:], in_=wqk_d[:])
            nc.scalar.dma_start(out=wvim_all[:], in_=wvim_d[:])
            # the scalar queue is ALSO the ACT/exp queue: every DMA issue
            # parked there delays the first exp by ~650ns. It carries only
            # the early transfers (weights + 2 chunk-0 pieces, needed
            # before any exp can run anyway); chunks 1-3 ride sync alone.
            for c in range(NCH):
                for p_ in range(4):
                    eng = nc.scalar if (c == 0 and p_ % 2 == 1) else nc.sync
                    eng.dma_start(
                        out=xt_chunk(c, 2 * p_, 2 * p_ + 2),
                        in_=xb_d[:, c, 2 * p_ : 2 * p_ + 2, :],
                    )

            wqk_sb = [wqk_all[:, e * 2 * H : (e + 1) * 2 * H] for e in range(ET)]
            wv_sb = [wvim_all[:, e * H : (e + 1) * H] for e in range(ET)]
            iml_b = wvim_all[:, ET * H : ET * H + NIT]

            # ---- constants ----
            ident = cpool.tile([P, P], F32, tag="ident")
            make_identity(nc, ident[:])
            ident_b = cpool.tile([P, P], BF16, tag="ident_b")
            nc.vector.tensor_copy(ident_b[:], ident[:])
            # strict lower-triangular -1e30 (mask sT where j > i within block)
            ltri = cpool.tile([P, P], F32, tag="ltri")
            make_lower_triangular(nc, ltri[:], val=-1e30, diag=False)
            ltri_b = cpool.tile([P, P], BF16, tag="ltri_b")
            nc.vector.tensor_copy(ltri_b[:], ltri[:])

            # prewarm the ACT exp table off the critical path
            warm = cpool.tile([1, 2], F32, tag="warm")
            nc.gpsimd.memset(warm[:], 0.0)
            nc.scalar.activation(
                warm[:, 0:1], warm[:, 1:2], mybir.ActivationFunctionType.Exp
            )

            # PE warm-up: dummy matmuls on the identity flip the HAM clock
            # gate to 8/8 while the x DMA is still in flight.
            pwarm = ps_m.tile([P, P], F32, tag="pm", name="pwarm")
            for w in range(NWARM):
                nc.tensor.matmul(
                    pwarm[:], ident_b[:], ident_b[:], start=True, stop=True
                )

            # persistent per-chunk tiles; v holds 4 i-tiles x [128, 128]
            # (padded cols; col 64 of each block is the ones-column -> denom)
            qt_sb = [None] * NCH
            kt_sb = [None] * NCH
            v_sb = []
            for c in range(NCH):
                vt = vpool.tile([P, 4 * P], BF16, tag=f"v{c}")
                nc.gpsimd.memset(vt[:], 1.0)
                v_sb.append(vt)

            def v_blk(it):
                return v_sb[it // 4][:, (it % 4) * P : (it % 4) * P + P]

            def qt(c):
                return qt_sb[c][:]

            def kt_blk(j):
                return kt_sb[j // 4][:, (j % 4) * P : (j % 4 + 1) * P]

            # M[it] = ltri + diag(iml[:, it]): causal mask + masked-row boost
            m_sb = [None] * NIT
            po_sb = [None] * NCH

            def emit_mask(it):
                mt = mqpool.tile([P, P], BF16, tag="m", name=f"m{it}")
                nc.vector.scalar_tensor_tensor(
                    mt[:],
                    ident_b[:],
                    iml_b[:, it : it + 1],
                    ltri_b[:],
                    op0=mybir.AluOpType.mult,
                    op1=mybir.AluOpType.add,
                )
                m_sb[it] = mt

            def emit_proj_qk(c):
                # q/k projection (fused): psum[0:64]=qT, [64:128]=kT
                pqk = ps_m.tile([P, CH], F32, tag="pm", name="pqk")
                for e in range(ET):
                    nc.tensor.matmul(
                        pqk[:],
                        wqk_sb[e],
                        xt_sl(e, c),
                        start=(e == 0),
                        stop=(e == ET - 1),
                    )
                qtt = qkpool.tile([H, CH], BF16, tag=f"qt{c}")
                nc.vector.tensor_copy(qtt[:], pqk[0:H, :])
                qt_sb[c] = qtt
                ktt = qkpool.tile([H, CH], BF16, tag=f"kt{c}")
                nc.vector.tensor_copy(ktt[:], pqk[H : 2 * H, :])
                kt_sb[c] = ktt
                if c == 0:
                    # first two diag masks are needed by the very first
                    # score pair
                    emit_mask(0)
                    emit_mask(1)

            def emit_proj_v(c):
                # v projection (vT), then PE-transpose to v [t,h]
                pv = ps_m.tile([H, CH], F32, tag="pm", name="pv")
                for e in range(ET):
                    nc.tensor.matmul(
                        pv[:],
                        wv_sb[e],
                        xt_sl(e, c),
                        start=(e == 0),
                        stop=(e == ET - 1),
                    )
                vt_st = stpool.tile([H, CH], BF16, tag="vt")
                nc.vector.tensor_copy(vt_st[:], pv[:])
                pvt = ps_m.tile([P, 4 * H], BF16, tag="pm", name="pvt")
                for k in range(4):
                    nc.tensor.transpose(
                        pvt[:, k * H : (k + 1) * H],
                        vt_st[:, k * P : (k + 1) * P],
                        ident_b[0:H, 0:H],
                    )
                nc.vector.tensor_copy(
                    v_sb[c][:].rearrange("p (k w) -> p k w", w=P)[:, :, 0:H],
                    pvt[:].rearrange("p (k w) -> p k w", w=H),
                )
                # diag masks for this chunk's i-tiles (the c=0 first two are
                # emitted early in emit_proj_qk)
                for k in range(4):
                    it = c * 4 + k
                    if m_sb[it] is None:
                        emit_mask(it)

            def chunk_pairs(c):
                # full-tile pairs in natural order; the four diag tiles
                # are re-paired with the LARGER-offset tile in slot 0 so
                # the joint exp read window [offs0 .. 1024) has (almost)
                # no dead gap columns -- exp is the pacer of this phase.
                jd = 4 * c
                pairs = [(2 * m, 2 * m + 1) for m in range(2 * c)]
                if c == NCH - 1:
                    # keep i-tiles 12/13 finishing in the first diag pair
                    # so the poA half out path still overlaps the last pair
                    pairs += [(jd + 1, jd), (jd + 3, jd + 2)]
                else:
                    pairs += [(jd + 3, jd), (jd + 2, jd + 1)]
                return pairs

            def emit_s(c, pair, pss):
                # score matmuls for one j-pair of chunk c; returns offs
                offs = []
                for s, j in enumerate(pair):
                    off = max(0, j * P - c * CH)
                    offs.append(off)
                    full = j < 4 * c
                    nc.tensor.matmul(
                        pss[:, s * CH + off : (s + 1) * CH],
                        kt_blk(j),
                        qt(c)[:, off:CH],
                        start=True,
                        stop=full,
                    )
                    if not full:
                        nc.tensor.matmul(
                            pss[:, s * CH + off : s * CH + off + P],
                            ident_b[:],
                            m_sb[j][:],
                            start=False,
                            stop=True,
                        )
                return offs

            def emit_pv(c, pair, pt, offs, split_po, last_j, lastA_j=None):
                # ascending-j emission: j=0 (the start=True full-region
                # write) always lands before any accumulating overlap
                for s in sorted(range(2), key=lambda s_: pair[s_]):
                    j = pair[s]
                    off = offs[s]
                    if not split_po:
                        nc.tensor.matmul(
                            po_sb[c][:, off:CH],
                            v_blk(j),
                            pt[:, s * CH + off : (s + 1) * CH],
                            start=(j == 0),
                            stop=(j == last_j),
                        )
                    else:
                        HC = CH // 2
                        poA, poB = po_sb[c]
                        if off < HC:
                            nc.tensor.matmul(
                                poA[:, off:HC],
                                v_blk(j),
                                pt[:, s * CH + off : s * CH + HC],
                                start=(j == 0),
                                stop=(j == lastA_j),
                            )
                        ob_ = max(off, HC)
                        nc.tensor.matmul(
                            poB[:, ob_ - HC : HC],
                            v_blk(j),
                            pt[:, s * CH + ob_ : (s + 1) * CH],
                            start=(j == 0),
                            stop=(j == last_j),
                        )

            def emit_out_copy(c):
                ot = stpool.tile([HP, CH], BF16, tag="ot")
                nc.vector.tensor_copy(ot[:], po_sb[c][0:HP, :])
                return ot

            def emit_out_rest(c, ot):
                # transpose 4 i-tiles into one psum tile, then batched
                # normalize (masked rows come out as v via the LARGE boost).
                # pot lives in ps_o (not ps_m) so the next chunk's
                # projection psum never waits on this out-path's reads.
                pot = ps_o.tile([P, 4 * HP], BF16, tag="po", name="pot")
                for k in range(4):
                    nc.tensor.transpose(
                        pot[:, k * HP : (k + 1) * HP],
                        ot[:, k * P : (k + 1) * P],
                        ident_b[0:HP, 0:HP],
                    )
                pot3 = pot[:].rearrange("p (k w) -> p k w", w=HP)
                recip = blpool.tile([P, 4], F32, tag="recip")
                nc.vector.reciprocal(
                    recip[:].rearrange("p (k o) -> p k o", o=1),
                    pot3[:, :, H : H + 1],
                )
                ob = blpool.tile([P, 4 * H], F32, tag="ob")
                for k in range(4):
                    nc.vector.tensor_scalar_mul(
                        ob[:, k * H : (k + 1) * H],
                        pot3[:, k, 0:H],
                        recip[:, k : k + 1],
                    )
                nc.sync.dma_start(
                    out=out_d.rearrange("(n p) h -> p n h", p=P)[
                        :, c * 4 : (c + 1) * 4, :
                    ],
                    in_=ob[:].rearrange("p (n h) -> p n h", h=H),
                )

            def emit_out_last_half(c, po_half, h0):
                # out path for i-tiles [c*4+h0*2, c*4+h0*2+2) of the last
                # chunk (po was split in two psum tiles)
                ot = stpool.tile([HP, CH // 2], BF16, tag="oth")
                nc.vector.tensor_copy(ot[:], po_half[0:HP, :])
                pot = ps_m.tile([P, 2 * HP], BF16, tag="pm", name=f"poth{h0}")
                for k in range(2):
                    nc.tensor.transpose(
                        pot[:, k * HP : (k + 1) * HP],
                        ot[:, k * P : (k + 1) * P],
                        ident_b[0:HP, 0:HP],
                    )
                pot3 = pot[:].rearrange("p (k w) -> p k w", w=HP)
                recip = blpool.tile([P, 2], F32, tag="reciph")
                nc.vector.reciprocal(
                    recip[:].rearrange("p (k o) -> p k o", o=1),
                    pot3[:, :, H : H + 1],
                )
                ob = blpool.tile([P, 2 * H], F32, tag="obh")
                for k in range(2):
                    nc.vector.tensor_scalar_mul(
                        ob[:, k * H : (k + 1) * H],
                        pot3[:, k, 0:H],
                        recip[:, k : k + 1],
                    )
                # the last half rides the scalar queue (exp is done by then)
                # so it doesn't serialize behind the first half's transfer
                eng = nc.scalar if h0 == 1 else nc.sync
                eng.dma_start(
                    out=out_d.rearrange("(n p) h -> p n h", p=P)[
                        :, c * 4 + h0 * 2 : c * 4 + h0 * 2 + 2, :
                    ],
                    in_=ob[:].rearrange("p (n h) -> p n h", h=H),
                )

            # ---- main loop (simple per-pair emission measured fastest;
            # skewed/interleaved variants consistently lost ~1-2us) ----
            # the (c-1) output path is emitted AFTER the chunk-c projection
            # so PE never stalls on the DVE copies
            # chunk-boundary drought fix: the next chunk's qk-projection is
            # emitted MID-pair-loop (so qt/kt are ready the moment the
            # boundary is crossed and ACT never starves waiting for the
            # first scores), and this chunk's v-projection is deferred into
            # its own pair loop (v(c) is only read by the last two pairs).
            emit_proj_qk(0)
            emit_proj_v(0)
            ot_prev = None
            for c in range(NCH):
                split_po = c == NCH - 1
                njt = 4 * c + 4
                if split_po:
                    # out_rest(c-1) must allocate its pot BEFORE poA/poB,
                    # else the ps_o slot rotation makes pot wait on poA's
                    # read which sits behind it in the in-order PE queue
                    if ot_prev is not None:
                        emit_out_rest(c - 1, ot_prev)
                        ot_prev = None
                    poA = ps_o.tile([P, CH // 2], F32, tag="po", name="poA")
                    poB = ps_o.tile([P, CH // 2], F32, tag="po", name="poB")
                    po_sb[c] = (poA, poB)
                else:
                    po_sb[c] = ps_o.tile([P, CH], F32, tag="po", name="po")
                pairs = chunk_pairs(c)
                last_j = 15 if split_po else 4 * c + 2
                # pv lags 1-2 pairs behind the scores: pv(k-d) sits AFTER
                # s(k) in the in-order PE queue, so its wait on exp never
                # delays the dispatch of the next scores. Depth 2 only for
                # the late chunks (their x landed long ago; a deep pend
                # behind DMA-gated work traps the pipeline).
                depth = 1 if c < 2 else 2
                pend = []
                for pi, pair in enumerate(pairs):
                    pss = ps_s.tile([P, 2 * CH], F32, tag="ps")
                    offs = emit_s(c, pair, pss)
                    pt = ptpool.tile([P, 2 * CH], BF16, tag="pt")
                    nc.scalar.activation(
                        pt[:, offs[0] :], pss[:, offs[0] :],
                        mybir.ActivationFunctionType.Exp,
                        scale=SCALE,
                    )
                    if pi == 0 and c >= 1:
                        # this chunk's v-projection: only its diag pairs
                        # read v(c), so it fills PE while ACT runs exps
                        emit_proj_v(c)
                    if pi == 1 and ot_prev is not None:
                        emit_out_rest(c - 1, ot_prev)
                        ot_prev = None
                    if split_po and pi == len(pairs) - 1:
                        # poA (i-tiles 12,13) needs pair (13,12)'s pv;
                        # drain after the last pair's s/exp so the poA
                        # transposes never delay the scores dispatch
                        while pend:
                            emit_pv(c, *pend.pop(0), split_po, last_j, 13)
                        emit_out_last_half(c, poA, 0)
                    elif len(pend) >= depth:
                        emit_pv(c, *pend.pop(0), split_po, last_j, 13)
                    pend.append((pair, pt, offs))
                    if pi == len(pairs) - 2 and c + 1 < NCH:
                        emit_proj_qk(c + 1)
                while pend:
                    emit_pv(c, *pend.pop(0), split_po, last_j, 13)
                if not split_po:
                    ot_prev = emit_out_copy(c)
                else:
                    emit_out_last_half(c, poB, 1)

    nc.compile()
    return nc


_NC_CACHE = None


def _get_nc():
    global _NC_CACHE
    if _NC_CACHE is None:
        _NC_CACHE = build_nc()
    return _NC_CACHE


def make_in_maps(x, Wq, Wk, Wv, lengths):
    wqk = np.concatenate(
        [np.asarray(Wq, dtype=np.float32), np.asarray(Wk, dtype=np.float32)],
        axis=1,
    )
    # [E, F] -> [P, ET*F]
    wqk_t = np.ascontiguousarray(
        wqk.reshape(ET, P, 2 * H).transpose(1, 0, 2).reshape(P, ET * 2 * H)
    ).astype(BF16_NP)
    wv_t = (
        np.asarray(Wv, dtype=np.float32)
        .reshape(ET, P, H)
        .transpose(1, 0, 2)
        .reshape(P, ET * H)
    )
    in_maps = []
    for b in range(B):
        # x[b] [T, E] -> xb [P, NCH, ET, CH]:  xb[p,c,e,w] = x[c*CH+w, e*P+p]
        xb = np.ascontiguousarray(
            np.asarray(x[b], dtype=np.float32)
            .reshape(NCH, CH, ET, P)
            .transpose(3, 0, 2, 1)
        ).astype(BF16_NP)
        mflat = (np.arange(T) < int(lengths[b])).astype(np.float32)
        iml = (1.0 - mflat.reshape(NIT, P).T) * LARGE  # [128, 16]
        wvim = np.ascontiguousarray(
            np.concatenate([wv_t, iml], axis=1)
        ).astype(BF16_NP)
        in_maps.append({"xb": xb, "wqk": wqk_t, "wvim": wvim})
    return in_maps


def run(x, Wq, Wk, Wv, lengths, trace=False):
    nc = _get_nc()
    in_maps = make_in_maps(x, Wq, Wk, Wv, lengths)
    res = run_bass_kernel_spmd(
        nc, in_maps, core_ids=list(range(B)), trace=trace
    )
    out = np.stack([res.results[b]["out"] for b in range(B)], axis=0)
    return out, res


def kernel(x, Wq, Wk, Wv, lengths):
    try:
        out, _ = run(x, Wq, Wk, Wv, lengths, trace=False)
    except Exception:
        # the device occasionally wedges (NRT_EXEC_UNIT_UNRECOVERABLE);
        # request a core reset and retry once
        import os

        os.environ["NEURON_RT_RESET_CORES"] = "1"
        out, _ = run(x, Wq, Wk, Wv, lengths, trace=False)
    return out


# revision 44
# speedup vs baseline: 1.0130x; 1.0007x over previous
"""Causal single-head attention with per-batch length masking, on 8 trn2 cores.

Problem: x[8,2048,1024] f32, Wq/Wk/Wv[1024,64] f32, lengths[8] int64.
  q,k,v = x@W*;  s = q@k^T (causal + length-pair mask, -inf);  s *= H^-0.5
  out = softmax(s) @ v          -> [8, 2048, 64] f32

Math note: for row i < len: every causal key j<=i is also valid, so the
pair-mask never bites -> plain causal softmax. For row i >= len: only the
diagonal survives -> out[i] = v[i]. Instead of blending, LARGE is added to
the diagonal score of masked rows (via the same bias-matmul that applies
the causal mask), so softmax itself collapses to ~1 on the diagonal and
out[i] = v[i] falls out of the normal normalize path.

Sharding: data-parallel over batch, one batch element per NeuronCore.

Perf structure (v3):
  - matmuls pipeline on the PE at ~N*0.417ns once deps are ready; levers
    are startup, dependency stalls (in-order engine queues!), and tail.
  - emission order is software-pipelined: pv(m) is emitted after s(m+1)
    so the in-order PE queue never head-of-line blocks on the ACT exp;
    the next chunk's projection is split (qk | v) around the last pv.
  - DMA issue split across Sync+Scalar HWDGE queues, ordered by need;
    mid-kernel DMAs stay on Sync (Scalar is the ACT/exp queue).
  - ~24 dummy ident matmuls flip the PE HAM clock gate during DMA wait.
  - po psum->sbuf copies run on ACT (Copy activation); DVE was the
    second-busiest engine.
  - last chunk: po split in half, out path per half + final pair split,
    so the post-matmul tail is short.
"""

import sys

import numpy as np
import ml_dtypes

try:
    import concourse.bass as bass  # noqa: F401
except ImportError:
    sys.path.insert(0, "/opt/trn_rl_repo")

import concourse.bass as bass  # noqa: F401
import concourse.mybir as mybir
import concourse.tile as tile
from concourse import bacc
from concourse.bass_utils import run_bass_kernel_spmd
from concourse.masks import make_identity, make_lower_triangular

F32 = mybir.dt.float32
BF16 = mybir.dt.bfloat16
BF16_NP = ml_dtypes.bfloat16

B, T, E, H = 8, 2048, 1024, 64
HP = H + 2       # rows of po read back (out + denom col)
P = 128          # partitions
CH = 512         # i-chunk width
ET = E // P      # 8 e-tiles
NCH = T // CH    # 4 chunks
NIT = T // P     # 16 i-tiles
SCALE = float(H) ** -0.5
LARGE = 600.0    # masked-row diagonal boost; exp(600/8)=3.7e32 stays finite
NWARM = 24       # dummy matmuls to flip the PE HAM clock gate


def build_nc():
    nc = bacc.Bacc(
        "TRN2",
        target_bir_lowering=False,
        debug=False,
        num_devices=B,
    )

    xb_d = nc.dram_tensor("xb", [P, NCH, ET, CH], BF16, kind="ExternalInput").ap()
    wqk_d = nc.dram_tensor("wqk", [P, ET * 2 * H], BF16, kind="ExternalInput").ap()
    # wv [P, ET*H] then iml [P, NIT] packed on the free axis
    wvim_d = nc.dram_tensor(
        "wvim", [P, ET * H + NIT], BF16, kind="ExternalInput"
    ).ap()
    out_d = nc.dram_tensor("out", [T, H], F32, kind="ExternalOutput").ap()

    with tile.TileContext(nc) as tc:
        with (
            tc.tile_pool(name="const", bufs=1) as cpool,
            tc.tile_pool(name="xt", bufs=1) as xtpool,
            tc.tile_pool(name="qk", bufs=1) as qkpool,
            tc.tile_pool(name="v", bufs=1) as vpool,
            tc.tile_pool(name="mq", bufs=16) as mqpool,
            tc.tile_pool(name="stage", bufs=4) as stpool,
            tc.tile_pool(name="pt", bufs=4) as ptpool,
            tc.tile_pool(name="blend", bufs=4) as blpool,
            tc.tile_pool(name="ps_s", bufs=2, space="PSUM") as ps_s,
            tc.tile_pool(name="ps_o", bufs=2, space="PSUM") as ps_o,
            tc.tile_pool(name="ps_m", bufs=2, space="PSUM") as ps_m,
        ):
            # ---- DMA: split across sync + scalar queues, ordered by when
            # the data is first consumed.
            wqk_all = cpool.tile([P, ET * 2 * H], BF16, tag="wqk")
            xt_all = xtpool.tile([P, NCH * ET * CH], BF16, tag="xt")
            wvim_all = cpool.tile([P, ET * H + NIT], BF16, tag="wvim")

            def xt_sl(e, c):
                return xt_all[:, (c * ET + e) * CH : (c * ET + e + 1) * CH]

            def xt_chunk(c, e0, e1):
                return xt_all[
                    :, (c * ET + e0) * CH : (c * ET + e1) * CH
                ].rearrange("p (n w) -> p n w", w=CH)

            # x streams in 2-e-tile (256KB) pieces alternating across both
            # HWDGE queues in global consumption order, so projections are
            # gated at fine grain and compute starts as early as possible
            # (early DMA BW paces the first half of the kernel).
            # weights ride the scalar ring so the sync ring's FIRST transfer
            # is x e0-e1 -- the first matmul gates on max(wqk, e01) landing
            # in parallel instead of their sum on one ring.
            nc.scalar.dma_start(out=wqk_all[:], in_=wqk_d[:])
            nc.scalar.dma_start(out=wvim_all[:], in_=wvim_d[:])
            # the scalar queue is ALSO the ACT/exp queue: every DMA issue
            # parked there delays the first exp by ~650ns. It carries only
            # the early transfers (weights + 2 chunk-0 pieces, needed
            # before any exp can run anyway); chunks 1-3 ride sync alone.
            for c in range(2):
                for p_ in range(4):
                    eng = nc.scalar if (c == 0 and p_ % 2 == 1) else nc.sync
                    eng.dma_start(
                        out=xt_chunk(c, 2 * p_, 2 * p_ + 2),
                        in_=xb_d[:, c, 2 * p_ : 2 * p_ + 2, :],
                    )
            # chunks 2-3 have deadline slack: one transfer each, so their
            # descriptors enqueue ~3us earlier (6 fewer ~650ns serial
            # DMA-issue instructions on the sync queue)
            for c in range(2, NCH):
                nc.sync.dma_start(
                    out=xt_chunk(c, 0, 8), in_=xb_d[:, c, :, :]
                )

            wqk_sb = [wqk_all[:, e * 2 * H : (e + 1) * 2 * H] for e in range(ET)]
            wv_sb = [wvim_all[:, e * H : (e + 1) * H] for e in range(ET)]
            iml_b = wvim_all[:, ET * H : ET * H + NIT]

            # ---- constants ----
            ident = cpool.tile([P, P], F32, tag="ident")
            make_identity(nc, ident[:])
            ident_b = cpool.tile([P, P], BF16, tag="ident_b")
            nc.vector.tensor_copy(ident_b[:], ident[:])
            # strict lower-triangular -1e30 (mask sT where j > i within block)
            ltri = cpool.tile([P, P], F32, tag="ltri")
            make_lower_triangular(nc, ltri[:], val=-1e30, diag=False)
            ltri_b = cpool.tile([P, P], BF16, tag="ltri_b")
            nc.vector.tensor_copy(ltri_b[:], ltri[:])

            # prewarm the ACT exp table off the critical path
            warm = cpool.tile([1, 2], F32, tag="warm")
            nc.gpsimd.memset(warm[:], 0.0)
            nc.scalar.activation(
                warm[:, 0:1], warm[:, 1:2], mybir.ActivationFunctionType.Exp
            )

            # PE warm-up: dummy matmuls on the identity flip the HAM clock
            # gate to 8/8 while the x DMA is still in flight.
            pwarm = ps_m.tile([P, P], F32, tag="pm", name="pwarm")
            for w in range(NWARM):
                nc.tensor.matmul(
                    pwarm[:], ident_b[:], ident_b[:], start=True, stop=True
                )

            # persistent per-chunk tiles; v holds 4 i-tiles x [128, 128]
            # (padded cols; col 64 of each block is the ones-column -> denom)
            qt_sb = [None] * NCH
            kt_sb = [None] * NCH
            v_sb = []
            for c in range(NCH):
                vt = vpool.tile([P, 4 * P], BF16, tag=f"v{c}")
                nc.gpsimd.memset(vt[:], 1.0)
                v_sb.append(vt)

            def v_blk(it):
                return v_sb[it // 4][:, (it % 4) * P : (it % 4) * P + P]

            def qt(c):
                return qt_sb[c][:]

            def kt_blk(j):
                return kt_sb[j // 4][:, (j % 4) * P : (j % 4 + 1) * P]

            # M[it] = ltri + diag(iml[:, it]): causal mask + masked-row boost
            m_sb = [None] * NIT
            po_sb = [None] * NCH

            def emit_mask(it):
                mt = mqpool.tile([P, P], BF16, tag="m", name=f"m{it}")
                nc.vector.scalar_tensor_tensor(
                    mt[:],
                    ident_b[:],
                    iml_b[:, it : it + 1],
                    ltri_b[:],
                    op0=mybir.AluOpType.mult,
                    op1=mybir.AluOpType.add,
                )
                m_sb[it] = mt

            def emit_proj_qk(c):
                # q/k projection (fused): psum[0:64]=qT, [64:128]=kT
                pqk = ps_m.tile([P, CH], F32, tag="pm", name="pqk")
                for e in range(ET):
                    nc.tensor.matmul(
                        pqk[:],
                        wqk_sb[e],
                        xt_sl(e, c),
                        start=(e == 0),
                        stop=(e == ET - 1),
                    )
                qtt = qkpool.tile([H, CH], BF16, tag=f"qt{c}")
                nc.vector.tensor_copy(qtt[:], pqk[0:H, :])
                qt_sb[c] = qtt
                ktt = qkpool.tile([H, CH], BF16, tag=f"kt{c}")
                nc.vector.tensor_copy(ktt[:], pqk[H : 2 * H, :])
                kt_sb[c] = ktt
                if c == 0:
                    # first two diag masks are needed by the very first
                    # score pair
                    emit_mask(0)
                    emit_mask(1)

            def emit_proj_v(c):
                # v projection (vT), then PE-transpose to v [t,h]
                pv = ps_m.tile([H, CH], F32, tag="pm", name="pv")
                for e in range(ET):
                    nc.tensor.matmul(
                        pv[:],
                        wv_sb[e],
                        xt_sl(e, c),
                        start=(e == 0),
                        stop=(e == ET - 1),
                    )
                vt_st = stpool.tile([H, CH], BF16, tag="vt")
                nc.vector.tensor_copy(vt_st[:], pv[:])
                pvt = ps_m.tile([P, 4 * H], BF16, tag="pm", name="pvt")
                for k in range(4):
                    nc.tensor.transpose(
                        pvt[:, k * H : (k + 1) * H],
                        vt_st[:, k * P : (k + 1) * P],
                        ident_b[0:H, 0:H],
                    )
                nc.vector.tensor_copy(
                    v_sb[c][:].rearrange("p (k w) -> p k w", w=P)[:, :, 0:H],
                    pvt[:].rearrange("p (k w) -> p k w", w=H),
                )
                # diag masks for this chunk's i-tiles (the c=0 first two are
                # emitted early in emit_proj_qk)
                for k in range(4):
                    it = c * 4 + k
                    if m_sb[it] is None:
                        emit_mask(it)

            def chunk_pairs(c):
                # full-tile pairs in natural order; the four diag tiles
                # are re-paired with the LARGER-offset tile in slot 0 so
                # the joint exp read window [offs0 .. 1024) has (almost)
                # no dead gap columns -- exp is the pacer of this phase.
                jd = 4 * c
                pairs = [(2 * m, 2 * m + 1) for m in range(2 * c)]
                if c == NCH - 1:
                    # keep i-tiles 12/13 finishing in the first diag pair
                    # so the poA half out path still overlaps the last pair
                    pairs += [(jd + 1, jd), (jd + 3, jd + 2)]
                else:
                    pairs += [(jd + 3, jd), (jd + 2, jd + 1)]
                return pairs

            def emit_s(c, pair, pss):
                # score matmuls for one j-pair of chunk c; returns offs
                offs = []
                for s, j in enumerate(pair):
                    off = max(0, j * P - c * CH)
                    offs.append(off)
                    full = j < 4 * c
                    nc.tensor.matmul(
                        pss[:, s * CH + off : (s + 1) * CH],
                        kt_blk(j),
                        qt(c)[:, off:CH],
                        start=True,
                        stop=full,
                    )
                    if not full:
                        nc.tensor.matmul(
                            pss[:, s * CH + off : s * CH + off + P],
                            ident_b[:],
                            m_sb[j][:],
                            start=False,
                            stop=True,
                        )
                return offs

            def emit_pv(c, pair, pt, offs, split_po, last_j, lastA_j=None):
                # ascending-j emission: j=0 (the start=True full-region
                # write) always lands before any accumulating overlap
                for s in sorted(range(2), key=lambda s_: pair[s_]):
                    j = pair[s]
                    off = offs[s]
                    if not split_po:
                        nc.tensor.matmul(
                            po_sb[c][:, off:CH],
                            v_blk(j),
                            pt[:, s * CH + off : (s + 1) * CH],
                            start=(j == 0),
                            stop=(j == last_j),
                        )
                    else:
                        HC = CH // 2
                        poA, poB = po_sb[c]
                        if off < HC:
                            nc.tensor.matmul(
                                poA[:, off:HC],
                                v_blk(j),
                                pt[:, s * CH + off : s * CH + HC],
                                start=(j == 0),
                                stop=(j == lastA_j),
                            )
                        ob_ = max(off, HC)
                        nc.tensor.matmul(
                            poB[:, ob_ - HC : HC],
                            v_blk(j),
                            pt[:, s * CH + ob_ : (s + 1) * CH],
                            start=(j == 0),
                            stop=(j == last_j),
                        )

            def emit_out_copy(c):
                ot = stpool.tile([HP, CH], BF16, tag="ot")
                nc.vector.tensor_copy(ot[:], po_sb[c][0:HP, :])
                return ot

            def emit_out_rest(c, ot):
                # transpose 4 i-tiles into one psum tile, then batched
                # normalize (masked rows come out as v via the LARGE boost).
                # pot lives in ps_o (not ps_m) so the next chunk's
                # projection psum never waits on this out-path's reads.
                pot = ps_o.tile([P, 4 * HP], BF16, tag="po", name="pot")
                for k in range(4):
                    nc.tensor.transpose(
                        pot[:, k * HP : (k + 1) * HP],
                        ot[:, k * P : (k + 1) * P],
                        ident_b[0:HP, 0:HP],
                    )
                pot3 = pot[:].rearrange("p (k w) -> p k w", w=HP)
                recip = blpool.tile([P, 4], F32, tag="recip")
                nc.vector.reciprocal(
                    recip[:].rearrange("p (k o) -> p k o", o=1),
                    pot3[:, :, H : H + 1],
                )
                ob = blpool.tile([P, 4 * H], F32, tag="ob")
                for k in range(4):
                    nc.vector.tensor_scalar_mul(
                        ob[:, k * H : (k + 1) * H],
                        pot3[:, k, 0:H],
                        recip[:, k : k + 1],
                    )
                nc.sync.dma_start(
                    out=out_d.rearrange("(n p) h -> p n h", p=P)[
                        :, c * 4 : (c + 1) * 4, :
                    ],
                    in_=ob[:].rearrange("p (n h) -> p n h", h=H),
                )

            def emit_out_last_half(c, po_half, h0):
                # out path for i-tiles [c*4+h0*2, c*4+h0*2+2) of the last
                # chunk (po was split in two psum tiles)
                ot = stpool.tile([HP, CH // 2], BF16, tag="oth")
                nc.vector.tensor_copy(ot[:], po_half[0:HP, :])
                pot = ps_m.tile([P, 2 * HP], BF16, tag="pm", name=f"poth{h0}")
                for k in range(2):
                    nc.tensor.transpose(
                        pot[:, k * HP : (k + 1) * HP],
                        ot[:, k * P : (k + 1) * P],
                        ident_b[0:HP, 0:HP],
                    )
                pot3 = pot[:].rearrange("p (k w) -> p k w", w=HP)
                recip = blpool.tile([P, 2], F32, tag="reciph")
                nc.vector.reciprocal(
                    recip[:].rearrange("p (k o) -> p k o", o=1),
                    pot3[:, :, H : H + 1],
                )
                ob = blpool.tile([P, 2 * H], F32, tag="obh")
                for k in range(2):
                    nc.vector.tensor_scalar_mul(
                        ob[:, k * H : (k + 1) * H],
                        pot3[:, k, 0:H],
                        recip[:, k : k + 1],
                    )
                # the last half rides the scalar queue (exp is done by then)
                # so it doesn't serialize behind the first half's transfer
                eng = nc.scalar if h0 == 1 else nc.sync
                eng.dma_start(
                    out=out_d.rearrange("(n p) h -> p n h", p=P)[
                        :, c * 4 + h0 * 2 : c * 4 + h0 * 2 + 2, :
                    ],
                    in_=ob[:].rearrange("p (n h) -> p n h", h=H),
                )

            # ---- main loop (simple per-pair emission measured fastest;
            # skewed/interleaved variants consistently lost ~1-2us) ----
            # the (c-1) output path is emitted AFTER the chunk-c projection
            # so PE never stalls on the DVE copies
            # chunk-boundary drought fix: the next chunk's qk-projection is
            # emitted MID-pair-loop (so qt/kt are ready the moment the
            # boundary is crossed and ACT never starves waiting for the
            # first scores), and this chunk's v-projection is deferred into
            # its own pair loop (v(c) is only read by the last two pairs).
            emit_proj_qk(0)
            emit_proj_v(0)
            ot_prev = None
            for c in range(NCH):
                split_po = c == NCH - 1
                njt = 4 * c + 4
                if split_po:
                    # out_rest(c-1) must allocate its pot BEFORE poA/poB,
                    # else the ps_o slot rotation makes pot wait on poA's
                    # read which sits behind it in the in-order PE queue
                    if ot_prev is not None:
                        emit_out_rest(c - 1, ot_prev)
                        ot_prev = None
                    poA = ps_o.tile([P, CH // 2], F32, tag="po", name="poA")
                    poB = ps_o.tile([P, CH // 2], F32, tag="po", name="poB")
                    po_sb[c] = (poA, poB)
                else:
                    po_sb[c] = ps_o.tile([P, CH], F32, tag="po", name="po")
                pairs = chunk_pairs(c)
                last_j = 15 if split_po else 4 * c + 2
                # pv lags 1-2 pairs behind the scores: pv(k-d) sits AFTER
                # s(k) in the in-order PE queue, so its wait on exp never
                # delays the dispatch of the next scores. Depth 2 only for
                # the late chunks (their x landed long ago; a deep pend
                # behind DMA-gated work traps the pipeline).
                depth = 1 if c < 2 else 2
                pend = []
                for pi, pair in enumerate(pairs):
                    pss = ps_s.tile([P, 2 * CH], F32, tag="ps")
                    offs = emit_s(c, pair, pss)
                    pt = ptpool.tile([P, 2 * CH], BF16, tag="pt")
                    nc.scalar.activation(
                        pt[:, offs[0] :], pss[:, offs[0] :],
                        mybir.ActivationFunctionType.Exp,
                        scale=SCALE,
                    )
                    if pi == 0 and c >= 1:
                        # this chunk's v-projection: only its diag pairs
                        # read v(c), so it fills PE while ACT runs exps
                        emit_proj_v(c)
                    if pi == 1 and ot_prev is not None:
                        emit_out_rest(c - 1, ot_prev)
                        ot_prev = None
                    if split_po and pi == len(pairs) - 1:
                        # poA (i-tiles 12,13) needs pair (13,12)'s pv;
                        # drain after the last pair's s/exp so the poA
                        # transposes never delay the scores dispatch
                        while pend:
                            emit_pv(c, *pend.pop(0), split_po, last_j, 13)
                        emit_out_last_half(c, poA, 0)
                    elif len(pend) >= depth:
                        emit_pv(c, *pend.pop(0), split_po, last_j, 13)
                    pend.append((pair, pt, offs))
                    if pi == len(pairs) - 2 and c + 1 < NCH:
                        emit_proj_qk(c + 1)
                while pend:
                    emit_pv(c, *pend.pop(0), split_po, last_j, 13)
                if not split_po:
                    ot_prev = emit_out_copy(c)
                else:
                    emit_out_last_half(c, poB, 1)

    nc.compile()
    return nc


_NC_CACHE = None


def _get_nc():
    global _NC_CACHE
    if _NC_CACHE is None:
        _NC_CACHE = build_nc()
    return _NC_CACHE


def make_in_maps(x, Wq, Wk, Wv, lengths):
    wqk = np.concatenate(
        [np.asarray(Wq, dtype=np.float32), np.asarray(Wk, dtype=np.float32)],
        axis=1,
    )
    # [E, F] -> [P, ET*F]
    wqk_t = np.ascontiguousarray(
        wqk.reshape(ET, P, 2 * H).transpose(1, 0, 2).reshape(P, ET * 2 * H)
    ).astype(BF16_NP)
    wv_t = (
        np.asarray(Wv, dtype=np.float32)
        .reshape(ET, P, H)
        .transpose(1, 0, 2)
        .reshape(P, ET * H)
    )
    in_maps = []
    for b in range(B):
        # x[b] [T, E] -> xb [P, NCH, ET, CH]:  xb[p,c,e,w] = x[c*CH+w, e*P+p]
        xb = np.ascontiguousarray(
            np.asarray(x[b], dtype=np.float32)
            .reshape(NCH, CH, ET, P)
            .transpose(3, 0, 2, 1)
        ).astype(BF16_NP)
        mflat = (np.arange(T) < int(lengths[b])).astype(np.float32)
        iml = (1.0 - mflat.reshape(NIT, P).T) * LARGE  # [128, 16]
        wvim = np.ascontiguousarray(
            np.concatenate([wv_t, iml], axis=1)
        ).astype(BF16_NP)
        in_maps.append({"xb": xb, "wqk": wqk_t, "wvim": wvim})
    return in_maps


def run(x, Wq, Wk, Wv, lengths, trace=False):
    nc = _get_nc()
    in_maps = make_in_maps(x, Wq, Wk, Wv, lengths)
    res = run_bass_kernel_spmd(
        nc, in_maps, core_ids=list(range(B)), trace=trace
    )
    out = np.stack([res.results[b]["out"] for b in range(B)], axis=0)
    return out, res


def kernel(x, Wq, Wk, Wv, lengths):
    try:
        out, _ = run(x, Wq, Wk, Wv, lengths, trace=False)
    except Exception:
        # the device occasionally wedges (NRT_EXEC_UNIT_UNRECOVERABLE);
        # request a core reset and retry once
        import os

        os.environ["NEURON_RT_RESET_CORES"] = "1"
        out, _ = run(x, Wq, Wk, Wv, lengths, trace=False)
    return out
